# Initial kernel scaffold
#
"""Trainium2 Bass kernel for GQA attention (nn_Attention_40364102648437).

Problem: B=2, S=2048, HIDDEN=896, 14 q heads / 2 kv heads, head_dim 64,
RoPE (theta 1e6), causal softmax, o-projection.

Sharding (8 cores, SPMD): core = b*4 + kv*2 + half.
Each core owns one batch b, one kv head, and 4 q-head slots (7 q heads per
kv group are split 4+3; the last slot of the second half is a duplicate
whose wo rows are zeroed so its contribution vanishes). Every core computes
a full [S, HIDDEN] partial output (its heads' contribution through wo);
the host sums the 4 partials per batch.

On-core layout: everything is kept "transposed" ([feature, seq]) so the
PE contractions line up without on-chip transposes of activations:
  qT/kT tiles [128, S] hold two head-slots stacked (rows 0-63 / 64-127),
  scores are computed as S^T [k_pos, q_pos] via row-paired K=64 matmuls
  (tile_position rows 0/64 run concurrently in the PE array),
  exp() runs on ScalarE (no max subtraction -- scores are O(1) by
  construction), V is transposed on the PE once per k-block and augmented
  with a ones column so each PV matmul also produces the softmax
  denominator, and the o-projection consumes the [feature, seq] attention
  output directly as the stationary operand.
"""

import numpy as np

import concourse.bass as bass
import concourse.mybir as mybir
from concourse import bacc
from concourse.tile import TileContext
from concourse.masks import make_identity
from concourse.bass_utils import run_bass_kernel_spmd

F32 = mybir.dt.float32
MM_DT = mybir.dt.float32r  # matmul input dtype (full rate at free dim >= 256)

HIDDEN = 896
N_HEADS = 14
N_KV_HEADS = 2
HEAD_DIM = 64
B = 2
S = 2048
ROPE_THETA = 1000000.0
NH7 = HIDDEN // 128  # 7 hidden tiles
NKB = S // 128       # 16 key blocks
NJ = S // 256        # 8 query superblocks (256 q positions each)
MASK_VAL = -1e9


def _mm(ap):
    """Bitcast a fp32 AP to the matmul input dtype."""
    if MM_DT == F32:
        return ap
    return ap.bitcast(MM_DT)


def build_program():
    nc = bacc.Bacc("TRN2", target_bir_lowering=False, debug=False, num_devices=8)

    hsT = nc.dram_tensor("hsT", [HIDDEN, S], F32, kind="ExternalInput")
    wq4 = nc.dram_tensor("wq4", [HIDDEN, 256], F32, kind="ExternalInput")
    bq4 = nc.dram_tensor("bq4", [2, 128], F32, kind="ExternalInput")
    wkv = nc.dram_tensor("wkv", [HIDDEN, 128], F32, kind="ExternalInput")
    bkv = nc.dram_tensor("bkv", [1, 128], F32, kind="ExternalInput")
    wo4 = nc.dram_tensor("wo4", [256, HIDDEN], F32, kind="ExternalInput")
    cosd = nc.dram_tensor("cosd", [128, S], F32, kind="ExternalInput")
    sind = nc.dram_tensor("sind", [128, S], F32, kind="ExternalInput")
    maskD = nc.dram_tensor("maskD", [128, 1024], F32, kind="ExternalInput")
    out_d = nc.dram_tensor("out", [S, HIDDEN], F32, kind="ExternalOutput")

    EXP = mybir.ActivationFunctionType.Exp

    with TileContext(nc) as tc:
        with (
            tc.tile_pool(name="const", bufs=1) as cpool,
            tc.tile_pool(name="big", bufs=1) as bigpool,
        ):
            # ---- constants
            wkv_sb = cpool.tile([128, NH7 * 128], F32)
            for h in range(NH7):
                nc.sync.dma_start(
                    out=wkv_sb[:, h * 128 : (h + 1) * 128],
                    in_=wkv[h * 128 : (h + 1) * 128, :],
                )
            wq_sb = cpool.tile([128, NH7 * 256], F32)
            for h in range(NH7):
                nc.sync.dma_start(
                    out=wq_sb[:, h * 256 : (h + 1) * 256],
                    in_=wq4[h * 128 : (h + 1) * 128, :],
                )
            wo_sb = cpool.tile([128, 2 * HIDDEN], F32)
            for ft in range(2):
                nc.sync.dma_start(
                    out=wo_sb[:, ft * HIDDEN : (ft + 1) * HIDDEN],
                    in_=wo4[ft * 128 : (ft + 1) * 128, :],
                )
            cos_sb = cpool.tile([128, S], F32)
            nc.sync.dma_start(out=cos_sb[:], in_=cosd[:])
            sin_sb = cpool.tile([128, S], F32)
            nc.sync.dma_start(out=sin_sb[:], in_=sind[:])
            mask_sb = cpool.tile([128, 1024], F32)
            nc.sync.dma_start(out=mask_sb[:], in_=maskD[:])
            bq_sb = cpool.tile([128, 2], F32)
            nc.sync.dma_start(out=bq_sb[:], in_=bq4.rearrange("a p -> p a"))
            bkv_sb = cpool.tile([128, 1], F32)
            nc.sync.dma_start(out=bkv_sb[:], in_=bkv.rearrange("a p -> p a"))
            ident = cpool.tile([128, 128], F32)
            make_identity(nc, ident[:])
            ones_sb = cpool.tile([128, 64], F32)
            nc.vector.memset(ones_sb[:], 1.0)

            # ---- persistent activations
            qA = bigpool.tile([128, S], F32)
            qB = bigpool.tile([128, S], F32)
            kvT = bigpool.tile([128, S], F32)
            kdup = bigpool.tile([128, S], F32)
            v_sb = bigpool.tile([128, NKB * 65], F32)
            aoT0 = bigpool.tile([128, S], F32)
            aoT1 = bigpool.tile([128, S], F32)
            stg0 = bigpool.tile([64, S], F32)
            stg1 = bigpool.tile([64, S], F32)

            # ================= phase A: projections =================
            with (
                tc.tile_pool(name="hst", bufs=2) as hpool,
                tc.tile_pool(name="pps", bufs=2, space="PSUM") as ppool,
            ):
                for ss in range(4):
                    ssl = slice(ss * 512, (ss + 1) * 512)
                    hs_t = hpool.tile([128, NH7 * 512], F32)
                    nc.sync.dma_start(
                        out=hs_t[:],
                        in_=hsT[:, ssl].rearrange("(t p) n -> p (t n)", p=128),
                    )
                    kv_ps = ppool.tile([128, 512], F32)
                    for h in range(NH7):
                        nc.tensor.matmul(
                            kv_ps[:],
                            _mm(wkv_sb[:, h * 128 : (h + 1) * 128]),
                            _mm(hs_t[:, h * 512 : (h + 1) * 512]),
                            start=(h == 0),
                            stop=(h == NH7 - 1),
                        )
                    nc.vector.tensor_scalar_add(kvT[:, ssl], kv_ps[:], bkv_sb[:, 0:1])
                    for ft in range(2):
                        q_ps = ppool.tile([128, 512], F32)
                        for h in range(NH7):
                            nc.tensor.matmul(
                                q_ps[:],
                                _mm(wq_sb[:, h * 256 + ft * 128 : h * 256 + (ft + 1) * 128]),
                                _mm(hs_t[:, h * 512 : (h + 1) * 512]),
                                start=(h == 0),
                                stop=(h == NH7 - 1),
                            )
                        qt = (qA, qB)[ft]
                        nc.vector.tensor_scalar_add(
                            qt[:, ssl], q_ps[:], bq_sb[:, ft : ft + 1]
                        )

            # ---- duplicate kT into both partition halves
            nc.vector.tensor_copy(kdup[0:64, :], kvT[0:64, :])
            nc.sync.dma_start(out=kdup[64:128, :], in_=kvT[0:64, :])

            # ---- RoPE on qA, qB, kdup (cross-partition swap via DMA)
            with tc.tile_pool(name="swp", bufs=2) as swpool:
                for t in (qA, qB, kdup):
                    tsw = swpool.tile([128, S], F32)
                    for dst, src in ((0, 32), (32, 0), (64, 96), (96, 64)):
                        nc.sync.dma_start(
                            out=tsw[dst : dst + 32, :], in_=t[src : src + 32, :]
                        )
                    nc.vector.tensor_mul(tsw[:], tsw[:], sin_sb[:])
                    nc.vector.tensor_mul(t[:], t[:], cos_sb[:])
                    nc.vector.tensor_add(t[:], t[:], tsw[:])

            # ---- v natural layout [k_pos, 64] + ones column (col 64 of 65)
            nc.vector.memset(v_sb[:], 1.0)
            with tc.tile_pool(name="vtr", bufs=2, space="PSUM") as vpool:
                for kb in range(NKB):
                    vt_ps = vpool.tile([128, 64], F32)
                    nc.tensor.transpose(
                        vt_ps[:],
                        kvT[64:128, kb * 128 : (kb + 1) * 128],
                        ident[64:128, 64:128],
                    )
                    nc.vector.tensor_copy(v_sb[:, kb * 65 : kb * 65 + 64], vt_ps[:])

            # ================= phase B: attention =================
            with (
                tc.tile_pool(name="sps", bufs=2, space="PSUM") as spool,
                tc.tile_pool(name="ops", bufs=2, space="PSUM") as opool,
                tc.tile_pool(name="rps", bufs=2, space="PSUM") as rpool,
                tc.tile_pool(name="esb", bufs=3) as epool,
                tc.tile_pool(name="rcs", bufs=2) as rcpool,
            ):
                for pair in range(2):
                    qt = (qA, qB)[pair]
                    aoT = (aoT0, aoT1)[pair]
                    stg = (stg0, stg1)[pair]
                    for J in range(NJ):
                        qsl = slice(J * 256, (J + 1) * 256)
                        o_ab = opool.tile([65, 512], F32)
                        pend = None  # software pipeline: PV trails S^T/exp by 1
                        for g in range(J + 1):
                            s_ps = spool.tile([128, 1024], F32)
                            for i, kb in enumerate((2 * g, 2 * g + 1)):
                                for half in range(2):
                                    seg = (2 * i + half) * 256
                                    nc.tensor.matmul(
                                        s_ps[:, seg : seg + 256],
                                        _mm(kdup[half * 64 : (half + 1) * 64,
                                                 kb * 128 : (kb + 1) * 128]),
                                        _mm(qt[half * 64 : (half + 1) * 64, qsl]),
                                        start=True,
                                        stop=True,
                                    )
                            if g == J:
                                nc.vector.tensor_add(s_ps[:], s_ps[:], mask_sb[:])
                            e_sb = epool.tile([128, 1024], F32)
                            nc.scalar.activation(
                                e_sb[:], s_ps[:], EXP, bias=0.0, scale=0.125
                            )
                            if pend is not None:
                                _emit_pv(nc, o_ab, v_sb, *pend, J)
                            pend = (e_sb, g)
                        _emit_pv(nc, o_ab, v_sb, *pend, J)

                        # normalize: recip of sums, replicate across 64 rows, mul
                        rc = rcpool.tile([128, 512], F32)
                        rp = rpool.tile([64, 512], F32)
                        for sl in range(2):
                            csl = slice(sl * 256, (sl + 1) * 256)
                            nc.vector.reciprocal(rc[64:65, csl], o_ab[64:65, csl])
                            nc.tensor.matmul(
                                rp[:, csl],
                                _mm(ones_sb[64:65, :]),
                                _mm(rc[64:65, csl]),
                                start=True,
                                stop=True,
                            )
                            dst = aoT[0:64, qsl] if sl == 0 else stg[0:64, qsl]
                            nc.vector.tensor_mul(dst, o_ab[0:64, csl], rp[:, csl])
                    # move slot-b rows into partitions 64..127 of aoT
                    nc.sync.dma_start(out=aoT[64:128, :], in_=stg[0:64, :])

            # ================= phase C: output projection =================
            with (
                tc.tile_pool(name="fps", bufs=2, space="PSUM") as fpool,
                tc.tile_pool(name="osb", bufs=3) as obpool,
            ):
                for qb in range(NKB):
                    f_ps = fpool.tile([128, HIDDEN], F32)
                    for ft in range(2):
                        aoT = (aoT0, aoT1)[ft]
                        lhsT = aoT[:, qb * 128 : (qb + 1) * 128]
                        nc.tensor.matmul(
                            f_ps[:, 0:512],
                            _mm(lhsT),
                            _mm(wo_sb[:, ft * HIDDEN : ft * HIDDEN + 512]),
                            start=(ft == 0),
                            stop=(ft == 1),
                        )
                        nc.tensor.matmul(
                            f_ps[:, 512:HIDDEN],
                            _mm(lhsT),
                            _mm(wo_sb[:, ft * HIDDEN + 512 : (ft + 1) * HIDDEN]),
                            start=(ft == 0),
                            stop=(ft == 1),
                        )
                    ob = obpool.tile([128, HIDDEN], F32)
                    nc.vector.tensor_copy(ob[:], f_ps[:])
                    nc.sync.dma_start(
                        out=out_d[qb * 128 : (qb + 1) * 128, :], in_=ob[:]
                    )

    nc.compile()
    return nc


def _emit_pv(nc, o_ab, v_sb, e_sb, g, J):
    """PV accumulation for one exp'd group (k-blocks 2g, 2g+1)."""
    for i, kb in enumerate((2 * g, 2 * g + 1)):
        for sl in range(2):
            seg = (2 * i + sl) * 256
            nc.tensor.matmul(
                o_ab[:, sl * 256 : (sl + 1) * 256],
                _mm(v_sb[:, kb * 65 : (kb + 1) * 65]),
                _mm(e_sb[:, seg : seg + 256]),
                start=(g == 0 and i == 0),
                stop=(g == J and i == 1),
                skip_group_check=True,
            )


def _rope_tables():
    inv_freq = 1.0 / (ROPE_THETA ** (np.arange(0, HEAD_DIM, 2, dtype=np.float32) / HEAD_DIM))
    t = np.arange(S, dtype=np.float32)
    freqs = np.outer(t, inv_freq)  # [S, 32]
    emb = np.concatenate([freqs, freqs], axis=-1)  # [S, 64]
    cosT = np.cos(emb).T.astype(np.float32)  # [64, S]
    sinT = np.sin(emb).T.astype(np.float32)
    sinmod = sinT.copy()
    sinmod[0:32] = -sinmod[0:32]
    cosd = np.concatenate([cosT, cosT], axis=0)  # [128, S]
    sind = np.concatenate([sinmod, sinmod], axis=0)
    return np.ascontiguousarray(cosd), np.ascontiguousarray(sind)


def _masks():
    kp = np.arange(128)[:, None]
    qp = np.arange(128)[None, :]
    tri = np.where(kp <= qp, 0.0, MASK_VAL).astype(np.float32)  # [128,128]
    zeros = np.zeros((128, 128), np.float32)
    full = np.full((128, 128), MASK_VAL, np.float32)
    mask0 = np.concatenate([tri, zeros], axis=1)  # kb 2J vs [2J, 2J+1]
    mask1 = np.concatenate([full, tri], axis=1)   # kb 2J+1 vs [2J, 2J+1]
    return np.ascontiguousarray(
        np.concatenate([mask0, mask0, mask1, mask1], axis=1)
    )  # [128, 1024]


def make_in_maps(hidden_states, wq, bq, wk, bk, wv, bv, wo):
    cosd, sind = _rope_tables()
    maskD = _masks()
    in_maps = []
    for core in range(8):
        b, kv, half = core // 4, (core % 4) // 2, core % 2
        if half == 0:
            slots = [kv * 7 + 0, kv * 7 + 1, kv * 7 + 2, kv * 7 + 3]
            dup = []
        else:
            slots = [kv * 7 + 4, kv * 7 + 5, kv * 7 + 6, kv * 7 + 3]
            dup = [3]
        cols = np.concatenate(
            [np.arange(h * 64, (h + 1) * 64) for h in slots]
        )
        wq4 = np.ascontiguousarray(wq[:, cols])
        bq4 = np.ascontiguousarray(bq[cols].reshape(2, 128))
        wkv = np.ascontiguousarray(
            np.concatenate(
                [wk[:, kv * 64 : (kv + 1) * 64], wv[:, kv * 64 : (kv + 1) * 64]],
                axis=1,
            )
        )
        bkv = np.ascontiguousarray(
            np.concatenate(
                [bk[kv * 64 : (kv + 1) * 64], bv[kv * 64 : (kv + 1) * 64]]
            ).reshape(1, 128)
        )
        wo4 = wo[cols, :].copy()
        for d in dup:
            wo4[d * 64 : (d + 1) * 64, :] = 0.0
        in_maps.append(
            {
                "hsT": np.ascontiguousarray(hidden_states[b].T),
                "wq4": wq4,
                "bq4": bq4,
                "wkv": wkv,
                "bkv": bkv,
                "wo4": np.ascontiguousarray(wo4),
                "cosd": cosd,
                "sind": sind,
                "maskD": maskD,
            }
        )
    return in_maps


_NC_CACHE = None


def _get_program():
    global _NC_CACHE
    if _NC_CACHE is None:
        _NC_CACHE = build_program()
    return _NC_CACHE


def kernel(hidden_states, wq, bq, wk, bk, wv, bv, wo):
    hidden_states = np.asarray(hidden_states, np.float32)
    wq = np.asarray(wq, np.float32)
    bq = np.asarray(bq, np.float32)
    wk = np.asarray(wk, np.float32)
    bk = np.asarray(bk, np.float32)
    wv = np.asarray(wv, np.float32)
    bv = np.asarray(bv, np.float32)
    wo = np.asarray(wo, np.float32)

    nc = _get_program()
    in_maps = make_in_maps(hidden_states, wq, bq, wk, bk, wv, bv, wo)
    res = run_bass_kernel_spmd(nc, in_maps, list(range(8)))
    out = np.zeros((B, S, HIDDEN), np.float32)
    for core in range(8):
        out[core // 4] += res.results[core]["out"]
    return out


# revision 2
# speedup vs baseline: 1.0180x; 1.0180x over previous
"""Trainium2 Bass kernel for GQA attention (nn_Attention_40364102648437).

Problem: B=2, S=2048, HIDDEN=896, 14 q heads / 2 kv heads, head_dim 64,
RoPE (theta 1e6), causal softmax, o-projection.

Sharding (8 cores, SPMD): core = b*4 + kv*2 + half.
Each core owns one batch b, one kv head, and 4 q-head slots (7 q heads per
kv group are split 4+3; the last slot of the second half is a duplicate
whose wo rows are zeroed so its contribution vanishes). Every core computes
a full [S, HIDDEN] partial output (its heads' contribution through wo);
the host sums the 4 partials per batch.

On-core layout: everything is kept "transposed" ([feature, seq]) so the
PE contractions line up without on-chip transposes of activations:
  qT/kT tiles [128, S] hold two head-slots stacked (rows 0-63 / 64-127),
  scores are computed as S^T [k_pos, q_pos] via row-paired K=64 matmuls
  (tile_position rows 0/64 run concurrently in the PE array),
  exp() runs on ScalarE (no max subtraction -- scores are O(1) by
  construction), V is transposed on the PE once per k-block and augmented
  with a ones column so each PV matmul also produces the softmax
  denominator, and the o-projection consumes the [feature, seq] attention
  output directly as the stationary operand.
"""

import numpy as np

import concourse.bass as bass
import concourse.mybir as mybir
from concourse import bacc
from concourse.tile import TileContext
from concourse.masks import make_identity
from concourse.bass_utils import run_bass_kernel_spmd

F32 = mybir.dt.float32
MM_DT = mybir.dt.float32r  # matmul input dtype (full rate at free dim >= 256)

HIDDEN = 896
N_HEADS = 14
N_KV_HEADS = 2
HEAD_DIM = 64
B = 2
S = 2048
ROPE_THETA = 1000000.0
NH7 = HIDDEN // 128  # 7 hidden tiles
NKB = S // 128       # 16 key blocks
NJ = S // 256        # 8 query superblocks (256 q positions each)
MASK_VAL = -1e9


def _mm(ap):
    """Bitcast a fp32 AP to the matmul input dtype."""
    if MM_DT == F32:
        return ap
    return ap.bitcast(MM_DT)


def build_program():
    nc = bacc.Bacc("TRN2", target_bir_lowering=False, debug=False, num_devices=8)

    hsT = nc.dram_tensor("hsT", [HIDDEN, S], F32, kind="ExternalInput")
    wq4 = nc.dram_tensor("wq4", [HIDDEN, 256], F32, kind="ExternalInput")
    bq4 = nc.dram_tensor("bq4", [2, 128], F32, kind="ExternalInput")
    wkv = nc.dram_tensor("wkv", [HIDDEN, 128], F32, kind="ExternalInput")
    bkv = nc.dram_tensor("bkv", [1, 128], F32, kind="ExternalInput")
    wo4 = nc.dram_tensor("wo4", [256, HIDDEN], F32, kind="ExternalInput")
    cosd = nc.dram_tensor("cosd", [128, S], F32, kind="ExternalInput")
    sind = nc.dram_tensor("sind", [128, S], F32, kind="ExternalInput")
    maskD = nc.dram_tensor("maskD", [128, 1024], F32, kind="ExternalInput")
    out_d = nc.dram_tensor("out", [S, HIDDEN], F32, kind="ExternalOutput")

    EXP = mybir.ActivationFunctionType.Exp

    with TileContext(nc) as tc:
        with (
            tc.tile_pool(name="const", bufs=1) as cpool,
            tc.tile_pool(name="big", bufs=1) as bigpool,
        ):
            # ---- constants
            wkv_sb = cpool.tile([128, NH7 * 128], F32)
            for h in range(NH7):
                nc.sync.dma_start(
                    out=wkv_sb[:, h * 128 : (h + 1) * 128],
                    in_=wkv[h * 128 : (h + 1) * 128, :],
                )
            wq_sb = cpool.tile([128, NH7 * 256], F32)
            for h in range(NH7):
                nc.sync.dma_start(
                    out=wq_sb[:, h * 256 : (h + 1) * 256],
                    in_=wq4[h * 128 : (h + 1) * 128, :],
                )
            wo_sb = cpool.tile([128, 2 * HIDDEN], F32)
            for ft in range(2):
                nc.sync.dma_start(
                    out=wo_sb[:, ft * HIDDEN : (ft + 1) * HIDDEN],
                    in_=wo4[ft * 128 : (ft + 1) * 128, :],
                )
            cos_sb = cpool.tile([128, S], F32)
            nc.sync.dma_start(out=cos_sb[:], in_=cosd[:])
            sin_sb = cpool.tile([128, S], F32)
            nc.sync.dma_start(out=sin_sb[:], in_=sind[:])
            mask_sb = cpool.tile([128, 1024], F32)
            nc.sync.dma_start(out=mask_sb[:], in_=maskD[:])
            bq_sb = cpool.tile([128, 2], F32)
            nc.sync.dma_start(out=bq_sb[:], in_=bq4.rearrange("a p -> p a"))
            bkv_sb = cpool.tile([128, 1], F32)
            nc.sync.dma_start(out=bkv_sb[:], in_=bkv.rearrange("a p -> p a"))
            ident = cpool.tile([128, 128], F32)
            make_identity(nc, ident[:])
            ones_sb = cpool.tile([128, 64], F32)
            nc.vector.memset(ones_sb[:], 1.0)

            # ---- persistent activations
            qA = bigpool.tile([128, S], F32)
            qB = bigpool.tile([128, S], F32)
            kvT = bigpool.tile([128, S], F32)
            kdup = bigpool.tile([128, S], F32)
            v_sb = bigpool.tile([128, NKB * 65], F32)
            aoT0 = bigpool.tile([128, S], F32)
            aoT1 = bigpool.tile([128, S], F32)
            stg0 = bigpool.tile([64, S], F32)
            stg1 = bigpool.tile([64, S], F32)

            # ================= phase A: projections =================
            with (
                tc.tile_pool(name="hst", bufs=2) as hpool,
                tc.tile_pool(name="pps", bufs=2, space="PSUM") as ppool,
            ):
                for ss in range(4):
                    ssl = slice(ss * 512, (ss + 1) * 512)
                    hs_t = hpool.tile([128, NH7 * 512], F32)
                    nc.sync.dma_start(
                        out=hs_t[:].rearrange("p (t n) -> p t n", t=NH7),
                        in_=hsT[:, ssl].rearrange("(t p) n -> p t n", p=128),
                    )
                    kv_ps = ppool.tile([128, 512], F32)
                    for h in range(NH7):
                        nc.tensor.matmul(
                            kv_ps[:],
                            _mm(wkv_sb[:, h * 128 : (h + 1) * 128]),
                            _mm(hs_t[:, h * 512 : (h + 1) * 512]),
                            start=(h == 0),
                            stop=(h == NH7 - 1),
                        )
                    nc.vector.tensor_scalar_add(kvT[:, ssl], kv_ps[:], bkv_sb[:, 0:1])
                    for ft in range(2):
                        q_ps = ppool.tile([128, 512], F32)
                        for h in range(NH7):
                            nc.tensor.matmul(
                                q_ps[:],
                                _mm(wq_sb[:, h * 256 + ft * 128 : h * 256 + (ft + 1) * 128]),
                                _mm(hs_t[:, h * 512 : (h + 1) * 512]),
                                start=(h == 0),
                                stop=(h == NH7 - 1),
                            )
                        qt = (qA, qB)[ft]
                        nc.vector.tensor_scalar_add(
                            qt[:, ssl], q_ps[:], bq_sb[:, ft : ft + 1]
                        )

            # ---- duplicate kT into both partition halves
            nc.vector.tensor_copy(kdup[0:64, :], kvT[0:64, :])
            nc.sync.dma_start(out=kdup[64:128, :], in_=kvT[0:64, :])

            # ---- RoPE on qA, qB, kdup (cross-partition swap via DMA)
            with tc.tile_pool(name="swp", bufs=2) as swpool:
                for t in (qA, qB, kdup):
                    tsw = swpool.tile([128, S], F32)
                    for dst, src in ((0, 32), (32, 0), (64, 96), (96, 64)):
                        nc.sync.dma_start(
                            out=tsw[dst : dst + 32, :], in_=t[src : src + 32, :]
                        )
                    nc.vector.tensor_mul(tsw[:], tsw[:], sin_sb[:])
                    nc.vector.tensor_mul(t[:], t[:], cos_sb[:])
                    nc.vector.tensor_add(t[:], t[:], tsw[:])

            # ---- v natural layout [k_pos, 64] + ones column (col 64 of 65)
            nc.vector.memset(v_sb[:], 1.0)
            with tc.tile_pool(name="vtr", bufs=2, space="PSUM") as vpool:
                for kb in range(NKB):
                    vt_ps = vpool.tile([128, 64], F32)
                    nc.tensor.transpose(
                        vt_ps[:],
                        kvT[64:128, kb * 128 : (kb + 1) * 128],
                        ident[64:128, 64:128],
                    )
                    nc.vector.tensor_copy(v_sb[:, kb * 65 : kb * 65 + 64], vt_ps[:])

            # ================= phase B: attention =================
            with (
                tc.tile_pool(name="sps", bufs=2, space="PSUM") as spool,
                tc.tile_pool(name="ops", bufs=2, space="PSUM") as opool,
                tc.tile_pool(name="rps", bufs=2, space="PSUM") as rpool,
                tc.tile_pool(name="esb", bufs=3) as epool,
                tc.tile_pool(name="rcs", bufs=2) as rcpool,
            ):
                for pair in range(2):
                    qt = (qA, qB)[pair]
                    aoT = (aoT0, aoT1)[pair]
                    stg = (stg0, stg1)[pair]
                    for J in range(NJ):
                        qsl = slice(J * 256, (J + 1) * 256)
                        o_ab = opool.tile([65, 512], F32)
                        pend = None  # software pipeline: PV trails S^T/exp by 1
                        for g in range(J + 1):
                            s_ps = spool.tile([128, 1024], F32)
                            for i, kb in enumerate((2 * g, 2 * g + 1)):
                                for half in range(2):
                                    seg = (2 * i + half) * 256
                                    nc.tensor.matmul(
                                        s_ps[:, seg : seg + 256],
                                        _mm(kdup[half * 64 : (half + 1) * 64,
                                                 kb * 128 : (kb + 1) * 128]),
                                        _mm(qt[half * 64 : (half + 1) * 64, qsl]),
                                        start=True,
                                        stop=True,
                                    )
                            if g == J:
                                nc.vector.tensor_add(s_ps[:], s_ps[:], mask_sb[:])
                            e_sb = epool.tile([128, 1024], F32)
                            nc.scalar.activation(
                                e_sb[:], s_ps[:], EXP, bias=0.0, scale=0.125
                            )
                            if pend is not None:
                                _emit_pv(nc, o_ab, v_sb, *pend, J)
                            pend = (e_sb, g)
                        _emit_pv(nc, o_ab, v_sb, *pend, J)

                        # normalize: recip of sums, replicate across 64 rows, mul
                        rc = rcpool.tile([128, 512], F32)
                        rp = rpool.tile([64, 512], F32)
                        for sl in range(2):
                            csl = slice(sl * 256, (sl + 1) * 256)
                            nc.vector.reciprocal(rc[64:65, csl], o_ab[64:65, csl])
                            nc.tensor.matmul(
                                rp[:, csl],
                                _mm(ones_sb[64:65, :]),
                                _mm(rc[64:65, csl]),
                                start=True,
                                stop=True,
                            )
                            dst = aoT[0:64, qsl] if sl == 0 else stg[0:64, qsl]
                            nc.vector.tensor_mul(dst, o_ab[0:64, csl], rp[:, csl])
                    # move slot-b rows into partitions 64..127 of aoT
                    nc.sync.dma_start(out=aoT[64:128, :], in_=stg[0:64, :])

            # ================= phase C: output projection =================
            with (
                tc.tile_pool(name="fps", bufs=2, space="PSUM") as fpool,
                tc.tile_pool(name="osb", bufs=3) as obpool,
            ):
                for qb in range(NKB):
                    f_ps = fpool.tile([128, HIDDEN], F32)
                    for ft in range(2):
                        aoT = (aoT0, aoT1)[ft]
                        lhsT = aoT[:, qb * 128 : (qb + 1) * 128]
                        nc.tensor.matmul(
                            f_ps[:, 0:512],
                            _mm(lhsT),
                            _mm(wo_sb[:, ft * HIDDEN : ft * HIDDEN + 512]),
                            start=(ft == 0),
                            stop=(ft == 1),
                        )
                        nc.tensor.matmul(
                            f_ps[:, 512:HIDDEN],
                            _mm(lhsT),
                            _mm(wo_sb[:, ft * HIDDEN + 512 : (ft + 1) * HIDDEN]),
                            start=(ft == 0),
                            stop=(ft == 1),
                        )
                    ob = obpool.tile([128, HIDDEN], F32)
                    nc.vector.tensor_copy(ob[:], f_ps[:])
                    nc.sync.dma_start(
                        out=out_d[qb * 128 : (qb + 1) * 128, :], in_=ob[:]
                    )

    nc.compile()
    return nc


def _emit_pv(nc, o_ab, v_sb, e_sb, g, J):
    """PV accumulation for one exp'd group (k-blocks 2g, 2g+1)."""
    for i, kb in enumerate((2 * g, 2 * g + 1)):
        for sl in range(2):
            seg = (2 * i + sl) * 256
            nc.tensor.matmul(
                o_ab[:, sl * 256 : (sl + 1) * 256],
                _mm(v_sb[:, kb * 65 : (kb + 1) * 65]),
                _mm(e_sb[:, seg : seg + 256]),
                start=(g == 0 and i == 0),
                stop=(g == J and i == 1),
                skip_group_check=True,
            )


def _rope_tables():
    inv_freq = 1.0 / (ROPE_THETA ** (np.arange(0, HEAD_DIM, 2, dtype=np.float32) / HEAD_DIM))
    t = np.arange(S, dtype=np.float32)
    freqs = np.outer(t, inv_freq)  # [S, 32]
    emb = np.concatenate([freqs, freqs], axis=-1)  # [S, 64]
    cosT = np.cos(emb).T.astype(np.float32)  # [64, S]
    sinT = np.sin(emb).T.astype(np.float32)
    sinmod = sinT.copy()
    sinmod[0:32] = -sinmod[0:32]
    cosd = np.concatenate([cosT, cosT], axis=0)  # [128, S]
    sind = np.concatenate([sinmod, sinmod], axis=0)
    return np.ascontiguousarray(cosd), np.ascontiguousarray(sind)


def _masks():
    kp = np.arange(128)[:, None]
    qp = np.arange(128)[None, :]
    tri = np.where(kp <= qp, 0.0, MASK_VAL).astype(np.float32)  # [128,128]
    zeros = np.zeros((128, 128), np.float32)
    full = np.full((128, 128), MASK_VAL, np.float32)
    mask0 = np.concatenate([tri, zeros], axis=1)  # kb 2J vs [2J, 2J+1]
    mask1 = np.concatenate([full, tri], axis=1)   # kb 2J+1 vs [2J, 2J+1]
    return np.ascontiguousarray(
        np.concatenate([mask0, mask0, mask1, mask1], axis=1)
    )  # [128, 1024]


def make_in_maps(hidden_states, wq, bq, wk, bk, wv, bv, wo):
    cosd, sind = _rope_tables()
    maskD = _masks()
    in_maps = []
    for core in range(8):
        b, kv, half = core // 4, (core % 4) // 2, core % 2
        if half == 0:
            slots = [kv * 7 + 0, kv * 7 + 1, kv * 7 + 2, kv * 7 + 3]
            dup = []
        else:
            slots = [kv * 7 + 4, kv * 7 + 5, kv * 7 + 6, kv * 7 + 3]
            dup = [3]
        cols = np.concatenate(
            [np.arange(h * 64, (h + 1) * 64) for h in slots]
        )
        wq4 = np.ascontiguousarray(wq[:, cols])
        bq4 = np.ascontiguousarray(bq[cols].reshape(2, 128))
        wkv = np.ascontiguousarray(
            np.concatenate(
                [wk[:, kv * 64 : (kv + 1) * 64], wv[:, kv * 64 : (kv + 1) * 64]],
                axis=1,
            )
        )
        bkv = np.ascontiguousarray(
            np.concatenate(
                [bk[kv * 64 : (kv + 1) * 64], bv[kv * 64 : (kv + 1) * 64]]
            ).reshape(1, 128)
        )
        wo4 = wo[cols, :].copy()
        for d in dup:
            wo4[d * 64 : (d + 1) * 64, :] = 0.0
        in_maps.append(
            {
                "hsT": np.ascontiguousarray(hidden_states[b].T),
                "wq4": wq4,
                "bq4": bq4,
                "wkv": wkv,
                "bkv": bkv,
                "wo4": np.ascontiguousarray(wo4),
                "cosd": cosd,
                "sind": sind,
                "maskD": maskD,
            }
        )
    return in_maps


_NC_CACHE = None


def _get_program():
    global _NC_CACHE
    if _NC_CACHE is None:
        _NC_CACHE = build_program()
    return _NC_CACHE


def kernel(hidden_states, wq, bq, wk, bk, wv, bv, wo):
    hidden_states = np.asarray(hidden_states, np.float32)
    wq = np.asarray(wq, np.float32)
    bq = np.asarray(bq, np.float32)
    wk = np.asarray(wk, np.float32)
    bk = np.asarray(bk, np.float32)
    wv = np.asarray(wv, np.float32)
    bv = np.asarray(bv, np.float32)
    wo = np.asarray(wo, np.float32)

    nc = _get_program()
    in_maps = make_in_maps(hidden_states, wq, bq, wk, bk, wv, bv, wo)
    res = run_bass_kernel_spmd(nc, in_maps, list(range(8)))
    out = np.zeros((B, S, HIDDEN), np.float32)
    for core in range(8):
        out[core // 4] += res.results[core]["out"]
    return out
